# revision 1
# baseline (speedup 1.0000x reference)
"""Trainium2 Bass kernel for nn_Attention_14740327760418 (RBF-kernel attention).

Sharding: tensor-parallel over the H=8 heads, one head per NeuronCore.
Each core computes its head's full attention plus its slice of the W_o
projection; the host sums the 8 partial [B,S,D] outputs.

Math per head h (GAMMA=1, no causal mask, dropout=0):
  xn   = LayerNorm(x) * ln_w                (ln_w folded into W on host)
  Q    = xn @ Wq_h ; K = xn @ Wk_h ; V = xn @ Wv_h
  attn = exp(-(q2[s] + k2[t] - 2 qk[s,t]))  (d2 >= 28 for this data -> the
                                             reference's max(d2,0) is dead)
       = exp(-q2[s]) * exp(2 qk[s,t] - k2[t])
  out_h = (attn @ V) @ Wo_h
The exp(-k2[t]) factor rides along as a per-partition bias of the scores
exp; exp(-q2[s]) is applied as a per-partition scale on the final PSUM.

Matmuls run in float32r (full PE rate at N>=256; ~2^-13 operand rounding);
q2/k2 ones-matmuls run in exact fp32.
"""

import sys

sys.path.insert(0, "/opt/trn_rl_repo")

import numpy as np

B, S, D, H, P = 4, 1024, 256, 8, 128
DC = D // P      # 2 chunks of the embedding dim
SC = S // P      # 8 chunks of the sequence dim
NB = 512         # matmul moving-operand block
SB = S // NB     # 2 sequence blocks
LN_EPS = 1e-5

_PROGRAM_CACHE = {}


def build_program(n_iters=None):
    """Build the per-core Bass program. n_iters wraps the whole compute in a
    device-side For_i loop (for timing); None emits the plain single-shot body."""
    import concourse.bass as bass  # noqa: F401
    import concourse.mybir as mybir
    from concourse import bacc
    from concourse.tile import TileContext
    from concourse.masks import make_identity

    F32 = mybir.dt.float32
    F32R = mybir.dt.float32r
    AF = mybir.ActivationFunctionType
    ALU = mybir.AluOpType
    AX = mybir.AxisListType

    nc = bacc.Bacc(None, target_bir_lowering=False)
    x = nc.declare_dram_parameter("x", [B, S, D], F32, isOutput=False)
    wq = nc.declare_dram_parameter("wq", [D, D], F32, isOutput=False)
    wk = nc.declare_dram_parameter("wk", [D, D], F32, isOutput=False)
    wv = nc.declare_dram_parameter("wv", [D, D], F32, isOutput=False)
    wo = nc.declare_dram_parameter("wo", [D, D], F32, isOutput=False)
    out = nc.declare_dram_parameter("out", [B, S, D], F32, isOutput=True)

    with TileContext(nc) as tc:
        from contextlib import ExitStack

        with ExitStack() as ctx:
            cpool = ctx.enter_context(tc.tile_pool(name="cpool", bufs=1))
            wpool = ctx.enter_context(tc.tile_pool(name="wpool", bufs=1))
            bpool = ctx.enter_context(tc.tile_pool(name="bpool", bufs=2))
            gpool = ctx.enter_context(tc.tile_pool(name="gpool", bufs=2))
            spool = ctx.enter_context(tc.tile_pool(name="spool", bufs=5))
            ps_tr = ctx.enter_context(tc.tile_pool(name="ps_tr", bufs=1, space="PSUM"))
            ps_mm = ctx.enter_context(tc.tile_pool(name="ps_mm", bufs=7, space="PSUM"))

            def setup_consts():
                ident = cpool.tile([P, P], F32, tag="ident")
                make_identity(nc, ident[:])
                ones = cpool.tile([P, 1], F32, tag="ones")
                nc.vector.memset(ones[:], 1.0)
                return ident, ones

            def setup_weights():
                """Weight loads (emitted after batch 0's x loads so they
                don't delay the LayerNorm critical path at kernel start)."""
                w_r = {}
                for name, dram in (("wq", wq), ("wk", wk), ("wv", wv), ("wo", wo)):
                    wf = spool.tile([P, DC, D], F32, tag="wload")
                    nc.sync.dma_start(wf[:],
                                      dram[:].rearrange("(dc p) e -> p dc e", p=P))
                    wr = wpool.tile([P, DC, D], F32R, tag=f"{name}_r")
                    nc.vector.tensor_copy(wr[:], wf[:])
                    w_r[name] = wr
                return w_r

            def batch_ln(b, ident):
                # --- Stage A: LayerNorm + transpose -> xnT [d, s] (fp32r) ---
                # var = E[x^2] - mu^2; rstd = exp(-0.5*ln(var+eps)) keeps every
                # ACT func in the exp/ln/square/copy table family. All per-row
                # stats for the s-chunks live in [P, n] tiles (col = chunk),
                # so the tiny scalar chain is one instruction per step.
                xnT = bpool.tile([P, DC, S], F32R, tag="xnT")
                xts = bpool.tile([P, SC, D], F32, tag="xts")
                vsum = spool.tile([P, SC], F32, tag="vsum")
                msum = spool.tile([P, SC], F32, tag="msum")
                for g in range(SC // 4):
                    gs = slice(g * 4, (g + 1) * 4)
                    for sc in range(g * 4, (g + 1) * 4):
                        nc.sync.dma_start(xts[:, sc, :], x[b, sc * P:(sc + 1) * P, :])
                        sq = spool.tile([P, D], F32, tag="sq")
                        nc.vector.scalar_tensor_tensor(sq[:], xts[:, sc, :], 1.0,
                                                       xts[:, sc, :], ALU.mult,
                                                       ALU.mult,
                                                       accum_out=vsum[:, sc:sc + 1])
                        nc.vector.tensor_reduce(msum[:, sc:sc + 1], xts[:, sc, :],
                                                AX.X, ALU.add)
                    mu = spool.tile([P, 4], F32, tag="mu")
                    nc.vector.tensor_scalar_mul(mu[:], msum[:, gs], 1.0 / D)
                    mu2 = spool.tile([P, 4], F32, tag="mu2")
                    nc.vector.tensor_mul(mu2[:], mu[:], mu[:])
                    m2e = spool.tile([P, 4], F32, tag="m2e")
                    nc.vector.tensor_scalar_add(m2e[:], mu2[:], -LN_EPS)
                    veps = spool.tile([P, 4], F32, tag="veps")
                    nc.vector.scalar_tensor_tensor(veps[:], vsum[:, gs], 1.0 / D,
                                                   m2e[:], ALU.mult, ALU.subtract)
                    vln = spool.tile([P, 4], F32, tag="vln")
                    nc.scalar.activation(vln[:], veps[:], AF.Ln)
                    rstd = spool.tile([P, 4], F32, tag="rstd")
                    nc.scalar.activation(rstd[:], vln[:], AF.Exp, scale=-0.5)
                    musr = spool.tile([P, 4], F32, tag="musr")
                    nc.vector.tensor_mul(musr[:], mu[:], rstd[:])
                    for j in range(4):
                        sc = g * 4 + j
                        nc.vector.tensor_scalar(xts[:, sc, :], xts[:, sc, :],
                                                rstd[:, j:j + 1], musr[:, j:j + 1],
                                                ALU.mult, ALU.subtract)
                    for dc in range(DC):
                        ptg = ps_mm.tile([P, NB], F32, tag="pmm")
                        for j in range(4):
                            sc = g * 4 + j
                            nc.tensor.transpose(
                                ptg[:, j * P:(j + 1) * P],
                                xts[:, sc, dc * P:(dc + 1) * P], ident[:])
                        nc.vector.tensor_copy(
                            xnT[:, dc, g * NB:(g + 1) * NB], ptg[:])
                return xnT

            def batch_proj(b, xnT, ones, w_r):
                # --- Stage B: projections; squares accumulate straight into
                # the chunk-summed [P, S] tiles (ACT square for chunk 0,
                # ACT square to scratch + gpsimd in-place add for chunk 1) ---
                qt = bpool.tile([P, DC, S], F32R, tag="qt")
                kt = bpool.tile([P, DC, S], F32R, tag="kt")
                vt = bpool.tile([P, SC, D], F32R, tag="vt")
                qt2s = bpool.tile([P, S], F32, tag="qt2s")
                kt2s = bpool.tile([P, S], F32, tag="kt2s")
                for dst, dsts, w in ((qt, qt2s, w_r["wq"]), (kt, kt2s, w_r["wk"])):
                    for eo in range(DC):
                        for sb in range(SB):
                            pp = ps_mm.tile([P, NB], F32, tag="pmm")
                            for ei in range(DC):
                                nc.tensor.matmul(
                                    pp[:], w[:, ei, eo * P:(eo + 1) * P],
                                    xnT[:, ei, sb * NB:(sb + 1) * NB],
                                    start=(ei == 0), stop=(ei == DC - 1))
                            nc.vector.tensor_copy(
                                dst[:, eo, sb * NB:(sb + 1) * NB], pp[:])
                            if eo == 0:
                                nc.scalar.activation(
                                    dsts[:, sb * NB:(sb + 1) * NB], pp[:], AF.Square)
                            else:
                                sq2 = spool.tile([P, NB], F32, tag="sq2")
                                nc.scalar.activation(sq2[:], pp[:], AF.Square)
                                nc.gpsimd.tensor_add(
                                    dsts[:, sb * NB:(sb + 1) * NB],
                                    dsts[:, sb * NB:(sb + 1) * NB], sq2[:])
                for tp in range(SC // 2):
                    pv = ps_mm.tile([P, NB], F32, tag="pmm")
                    for half in range(2):
                        t = 2 * tp + half
                        for ei in range(DC):
                            nc.tensor.matmul(
                                pv[:, half * D:(half + 1) * D],
                                xnT[:, ei, t * P:(t + 1) * P],
                                w_r["wv"][:, ei, :],
                                start=(ei == 0), stop=(ei == DC - 1))
                    nc.vector.tensor_copy(
                        vt[:, 2 * tp:2 * tp + 2, :],
                        pv[:].rearrange("p (h d) -> p h d", h=2))

                # --- Stage C: q2/k2 column vectors (exact fp32 ones-matmuls) ---
                negk2 = bpool.tile([P, SC], F32, tag="negk2")
                eq2 = bpool.tile([P, SC], F32, tag="eq2")
                for t in range(SC):
                    pk2 = ps_tr.tile([P, 1], F32, tag="pt")
                    nc.tensor.matmul(pk2[:], kt2s[:, t * P:(t + 1) * P],
                                     ones[:], start=True, stop=True)
                    nc.vector.tensor_scalar_mul(negk2[:, t:t + 1], pk2[:], -1.0)
                for sc in range(SC):
                    pq2 = ps_tr.tile([P, 1], F32, tag="pt")
                    nc.tensor.matmul(pq2[:], qt2s[:, sc * P:(sc + 1) * P],
                                     ones[:], start=True, stop=True)
                    nc.scalar.activation(eq2[:, sc:sc + 1], pq2[:], AF.Exp, scale=-1.0)

                return qt, kt, vt, negk2, eq2

            def batch_attn(b, proj, ones, w_r):
                qt, kt, vt, negk2, eq2 = proj
                # --- Stage D: scores -> exp -> attn @ V (transposed output) ---
                oT = bpool.tile([P, DC, S], F32R, tag="oT")
                for sb in range(SB):
                    gt = gpool.tile([P, SC, NB], F32R, tag="gt")
                    for t in range(SC):
                        pscr = ps_mm.tile([P, NB], F32, tag="pmm")
                        for ei in range(DC):
                            nc.tensor.matmul(pscr[:], kt[:, ei, t * P:(t + 1) * P],
                                             qt[:, ei, sb * NB:(sb + 1) * NB],
                                             start=(ei == 0), stop=(ei == DC - 1))
                        nc.scalar.activation(gt[:, t, :], pscr[:], AF.Exp,
                                             bias=negk2[:, t:t + 1], scale=2.0)
                    pos = [ps_mm.tile([P, NB], F32, tag="pmm", name=f"po{sb}_{ec2}")
                           for ec2 in range(DC)]
                    for t in range(SC):
                        for ec in range(DC):
                            nc.tensor.matmul(pos[ec][:], vt[:, t, ec * P:(ec + 1) * P],
                                             gt[:, t, :],
                                             start=(t == 0), stop=(t == SC - 1))
                    for ec in range(DC):
                        nc.vector.tensor_copy(oT[:, ec, sb * NB:(sb + 1) * NB],
                                              pos[ec][:])

                # --- Stage E: W_o projection + exp(-q2[s]) scale ---
                for sc in range(SC):
                    pf = ps_mm.tile([P, NB], F32, tag="pmm")
                    for ec in range(DC):
                        nc.tensor.matmul(pf[:, :D], oT[:, ec, sc * P:(sc + 1) * P],
                                         w_r["wo"][:, ec, :],
                                         start=(ec == 0), stop=(ec == DC - 1))
                    of = spool.tile([P, D], F32, tag="of")
                    nc.scalar.activation(of[:], pf[:, :D], AF.Copy,
                                         bias=0.0, scale=eq2[:, sc:sc + 1])
                    nc.sync.dma_start(out[b, sc * P:(sc + 1) * P, :], of[:])

            def full_body():
                # Plain sequential emission: Tile's dependency-driven scheduler
                # already overlaps the next batch's LayerNorm under the current
                # batch's matmuls; explicit reordering only distorts the
                # program-order priorities (measured worse in sim both ways).
                ident, ones = setup_consts()
                w_r = None
                for b in range(B):
                    xnT = batch_ln(b, ident)
                    if w_r is None:
                        w_r = setup_weights()
                    proj = batch_proj(b, xnT, ones, w_r)
                    batch_attn(b, proj, ones, w_r)

            if n_iters is None:
                full_body()
            else:
                with tc.For_i(0, n_iters, 1):
                    full_body()

    nc.compile()
    return nc


def _get_program(n_iters=None):
    key = n_iters
    if key not in _PROGRAM_CACHE:
        _PROGRAM_CACHE[key] = build_program(n_iters)
    return _PROGRAM_CACHE[key]


def make_in_maps(x, W_q, W_k, W_v, W_o, ln_w):
    x = np.ascontiguousarray(np.asarray(x, dtype=np.float32))
    lw = np.asarray(ln_w, dtype=np.float32)[:, None]
    maps = []
    for h in range(H):
        maps.append({
            "x": x,
            "wq": np.ascontiguousarray(lw * np.asarray(W_q[h], dtype=np.float32)),
            "wk": np.ascontiguousarray(lw * np.asarray(W_k[h], dtype=np.float32)),
            "wv": np.ascontiguousarray(lw * np.asarray(W_v[h], dtype=np.float32)),
            "wo": np.ascontiguousarray(
                np.asarray(W_o[h * D:(h + 1) * D, :], dtype=np.float32)),
        })
    return maps


def kernel(x, e, p, W_q, W_k, W_v, W_o, ln_w):
    from concourse.bass_utils import run_bass_kernel_spmd

    nc = _get_program()
    in_maps = make_in_maps(x, W_q, W_k, W_v, W_o, ln_w)
    res = run_bass_kernel_spmd(nc, in_maps, list(range(H)))
    total = np.zeros((B, S, D), dtype=np.float64)
    for r in res.results:
        total += r["out"].astype(np.float64)
    return total.astype(np.float32)

